# revision 12
# baseline (speedup 1.0000x reference)
"""Multi-head attention (B=2, T=2048, d_model=1024, H=16, hd=64) on 8 Trainium2
NeuronCores.

Sharding: 4 consecutive heads of one batch per core (core c -> batch c//4,
heads 4*(c%4)..+3). Each core: QKV projection slice, causal attention, partial
out-projection (its 256 rows of W_out); host sums 4 partials/batch + b_out.

v2 layout (everything stays transposed end-to-end; no on-chip transposes):
  qT/kT [hd, T]   <- lhsT=W chunk, rhs=xT (2 heads per 128 partitions)
  sT    [k, q]    <- two K=64 matmuls, one per head, row-paired on the PE
                     (tile_position rows 0/64) -> concurrent
  E     [k, q]    <- exp(sT/8) on ScalarE (bf16); diagonal blocks masked
                     post-exp with a 0/1 triangle; above-diagonal blocks never
                     computed (scores/exp/pv all narrowed to the causal band)
  pvT   [65, q]   <- lhsT=[v|1] chunk, rhs=E chunk: rows 0-63 unnormalized
                     aT, row 64 = softmax denominators (free)
  aT    [hd, q]   = pvT[0:64] * bcast(1/pvT[64]) (recip_approx_fast on the
                     psum row + stride-0 broadcast DMA + one DVE multiply)
  out  += aT.T @ W_out chunk  (psum -> DRAM DMA directly, fp32)

Emission is software-pipelined: head-pair 0's attention starts right after
its projections (pair-1 projections + v tiles ride along as fillers); pv lags
scores by one q-group so the PE never stalls on exp; out-projection for
q-group g is emitted inside head-pair 1's loop right after group g finishes.
"""

import math
import os
from contextlib import ExitStack
from dataclasses import dataclass

import numpy as np
import ml_dtypes

import concourse.bass as bass
import concourse.tile as tile
from concourse import bacc, mybir
from concourse import bass_utils

AF = mybir.ActivationFunctionType
ALU = mybir.AluOpType
DT = mybir.dt

N_CORES = 8


@dataclass(frozen=True)
class Cfg:
    T: int = 2048        # sequence length
    DM: int = 1024       # d_model
    HD: int = 64         # head dim
    NH: int = 4          # heads per core
    mode: str = "causal"
    mm: str = "bf16"
    dbg: bool = False

    @property
    def NHD(self):
        return self.NH * self.HD          # qkv slice width per core

    @property
    def KC(self):
        return self.DM // 128             # contraction chunks for projections

    @property
    def MC(self):
        return self.NHD // 128            # head-pair chunks

    @property
    def TC(self):
        return self.T // 128              # t chunks

    @property
    def QW(self):
        return 512                        # q group width

    @property
    def QG(self):
        return self.T // self.QW

    @property
    def QT(self):
        return self.QW // 128             # q tiles (128) per group

    @property
    def npmm(self):
        return ml_dtypes.bfloat16


def build_program(cfg: Cfg):
    c = cfg
    assert c.mode == "causal"
    assert c.DM == 1024 and c.T == 2048 and c.HD == 64 and c.NH == 4
    nc = bacc.Bacc("TRN2", target_bir_lowering=False, debug=False,
                   num_devices=N_CORES)
    f32 = DT.float32
    bf16 = DT.bfloat16

    # xT has an extra 128-row chunk: row 1024 = ones (for the v bias matmul)
    xT = nc.dram_tensor("xT", [c.DM + 128, c.T], bf16, kind="ExternalInput").ap()
    wq = nc.dram_tensor("wq", [c.DM, c.NHD], bf16, kind="ExternalInput").ap()
    wk = nc.dram_tensor("wk", [c.DM, c.NHD], bf16, kind="ExternalInput").ap()
    # wv has the bias as row 1024 (contracts against the ones row of xT)
    wv = nc.dram_tensor("wv", [c.DM + 128, c.NHD], bf16, kind="ExternalInput").ap()
    bq = nc.dram_tensor("bq", [128, c.MC], f32, kind="ExternalInput").ap()
    bk = nc.dram_tensor("bk", [128, c.MC], f32, kind="ExternalInput").ap()
    wo = nc.dram_tensor("wo", [c.NHD, c.DM], bf16, kind="ExternalInput").ap()
    out = nc.dram_tensor("out", [c.T, c.DM], bf16, kind="ExternalOutput").ap()
    dbg = {}
    if c.dbg:
        dbg["qT"] = nc.dram_tensor("dbg_qT", [128, c.MC, c.T], bf16, kind="ExternalOutput").ap()
        dbg["kT"] = nc.dram_tensor("dbg_kT", [128, c.MC, c.T], bf16, kind="ExternalOutput").ap()
        dbg["v"] = nc.dram_tensor("dbg_v", [128, c.TC, c.NH, 128], bf16, kind="ExternalOutput").ap()
        dbg["E00"] = nc.dram_tensor("dbg_E00", [128, 1024], bf16, kind="ExternalOutput").ap()
        dbg["psv"] = nc.dram_tensor("dbg_psv", [128, 512], DT.float32, kind="ExternalOutput").ap()
        dbg["rf"] = nc.dram_tensor("dbg_rf", [1, 512], DT.float32, kind="ExternalOutput").ap()
        dbg["rb"] = nc.dram_tensor("dbg_rb", [64, 512], DT.float32, kind="ExternalOutput").ap()
        dbg["aT"] = nc.dram_tensor("dbg_aT", [128, c.MC, c.T], bf16, kind="ExternalOutput").ap()

    with tile.TileContext(nc) as tc, ExitStack() as ctx:
        _body(ctx, tc, c, xT, wq, wk, wv, bq, bk, wo, out, dbg)
    nc.compile()
    return nc, ["xT", "wq", "wk", "wv", "bq", "bk", "wo"]


def _body(ctx, tc, c: Cfg, xT, wq, wk, wv, bq, bk, wo, out, dbg=None):
    nc = tc.nc
    f32 = DT.float32
    bf16 = DT.bfloat16
    scale = 1.0 / math.sqrt(c.HD)
    HD1 = c.HD + 1

    const = ctx.enter_context(tc.tile_pool(name="const", bufs=1))
    big = ctx.enter_context(tc.tile_pool(name="big", bufs=1))
    epool = ctx.enter_context(tc.tile_pool(name="E", bufs=28))
    rfpool = ctx.enter_context(tc.tile_pool(name="rf", bufs=2))
    rbpool = ctx.enter_context(tc.tile_pool(name="rb", bufs=2))
    stg = ctx.enter_context(tc.tile_pool(name="stg", bufs=2))
    # PSUM: 3 x [128,1024] (6 banks) + 2 x [65,512] (2 banks)
    ps_mm = ctx.enter_context(tc.tile_pool(name="psmm", bufs=3, space="PSUM"))
    ps_pv = ctx.enter_context(tc.tile_pool(name="pspv", bufs=2, space="PSUM"))

    # ---- input DMAs (ordered for earliest compute start) ----
    bq_sb = const.tile([128, c.MC], f32, tag="bq")
    nc.sync.dma_start(out=bq_sb[:], in_=bq)
    bk_sb = const.tile([128, c.MC], f32, tag="bk")
    nc.sync.dma_start(out=bk_sb[:], in_=bk)

    wq_sb = big.tile([128, c.KC, c.NHD], bf16, tag="wq")
    nc.sync.dma_start(out=wq_sb[:], in_=wq.rearrange("(c p) n -> p c n", p=128))
    wk_sb = big.tile([128, c.KC, c.NHD], bf16, tag="wk")
    nc.sync.dma_start(out=wk_sb[:], in_=wk.rearrange("(c p) n -> p c n", p=128))

    XC = c.KC + 1
    xT_sb = big.tile([128, XC, c.T], bf16, tag="xT")
    xTd = xT.rearrange("(c p) t -> p c t", p=128)
    TH = c.T // 2
    nc.sync.dma_start(out=xT_sb[:, :, 0:TH], in_=xTd[:, :, 0:TH])

    wv_sb = big.tile([128, XC, c.NHD], bf16, tag="wv")
    nc.sync.dma_start(out=wv_sb[:], in_=wv.rearrange("(c p) n -> p c n", p=128))

    nc.sync.dma_start(out=xT_sb[:, :, TH:c.T], in_=xTd[:, :, TH:c.T])

    wo_sb = big.tile([128, c.MC, c.DM], bf16, tag="wo")
    nc.sync.dma_start(out=wo_sb[:], in_=wo.rearrange("(c p) n -> p c n", p=128))

    # 0/1 lower-causal triangle: tri01[k, q] = 1 if q >= k else 0
    tri01 = const.tile([128, 128], bf16, tag="tri")
    nc.gpsimd.memset(tri01[:], 1.0)
    nc.gpsimd.affine_select(
        out=tri01[:], in_=tri01[:],
        compare_op=ALU.is_ge, fill=0.0,
        base=0, channel_multiplier=-1, pattern=[[1, 128]],
    )

    # ---- projection targets ----
    qT_sb = big.tile([128, c.MC, c.T], bf16, tag="qT")
    kT_sb = big.tile([128, c.MC, c.T], bf16, tag="kT")
    v_sb = big.tile([128, c.TC, c.NH, 128], bf16, tag="v")
    nc.vector.memset(v_sb[:, :, :, 0:c.HD], 0.0)
    nc.vector.memset(v_sb[:, :, :, 0:1], 1.0)
    aT_sb = big.tile([128, c.MC, c.T], bf16, tag="aT")

    W2 = 1024                              # q/k psum tile width (t cols)
    VG = 4                                 # t-chunks per v psum tile

    def emit_qk_tile(m, which, n):
        """One [128, 1024] psum tile of the q or k projection."""
        w_sb, b_sb, dst = ((wq_sb, bq_sb, qT_sb) if which == "q"
                           else (wk_sb, bk_sb, kT_sb))
        ps = ps_mm.tile([128, 1024], f32, tag="mm")
        for d in range(W2 // 512):
            for kc in range(c.KC):
                nc.tensor.matmul(
                    ps[:, d * 512:(d + 1) * 512],
                    lhsT=w_sb[:, kc, m * 128:(m + 1) * 128],
                    rhs=xT_sb[:, kc, n * W2 + d * 512:n * W2 + (d + 1) * 512],
                    start=(kc == 0), stop=(kc == c.KC - 1),
                )
        nc.vector.tensor_scalar_add(
            dst[:, m, n * W2:(n + 1) * W2], ps[:, 0:W2], b_sb[:, m:m + 1],
        )

    def emit_v_tile(tg):
        """VG t-chunks of v for all heads (+ bias via the ones row chunk)."""
        ps = ps_mm.tile([128, 1024], f32, tag="mm")
        for d in range(VG):
            t = tg * VG + d
            for kc in range(XC):
                nc.tensor.matmul(
                    ps[:, d * c.NHD:(d + 1) * c.NHD],
                    lhsT=xT_sb[:, kc, t * 128:(t + 1) * 128],
                    rhs=wv_sb[:, kc, :],
                    start=(kc == 0), stop=(kc == XC - 1),
                )
        for d in range(VG):
            t = tg * VG + d
            nc.vector.tensor_copy(
                v_sb[:, t, :, c.HD:128],
                ps[:, d * c.NHD:(d + 1) * c.NHD].rearrange(
                    "p (h d) -> p h d", d=c.HD),
            )

    # ---- attention ----
    # Per (hp, g): kc ranges over the causal band 0..4g+3. For band chunks
    # (kc >= 4g) only columns q >= kc*128 are computed anywhere: the scores
    # matmul, the exp, and the pv matmul are all narrowed. E tiles pack two
    # kc chunks [w(2kp) | w(2kp+1)] so one ACT covers both.
    def jstart(kc, g):
        return max(0, kc - c.QT * g)

    def attn_group(hp, g, etiles):
        """Scores + exp for (hp, g) -> etiles[(hl, kp)] = (tile, colbases)."""
        for kp in range((g + 1) * c.QT // 2):
            kcs = (2 * kp, 2 * kp + 1)
            js = [jstart(kc, g) for kc in kcs]
            ws = [512 - 128 * j for j in js]
            cb = [0, ws[0]]
            ps_h0 = ps_mm.tile([128, 1024], f32, tag="mm")
            ps_h1 = ps_mm.tile([128, 1024], f32, tag="mm")
            pss = [ps_h0, ps_h1]
            for d in range(2):
                for hl in range(2):
                    nc.tensor.matmul(
                        pss[hl][:, cb[d]:cb[d] + ws[d]],
                        lhsT=kT_sb[hl * 64:(hl + 1) * 64, hp,
                                   kcs[d] * 128:(kcs[d] + 1) * 128],
                        rhs=qT_sb[hl * 64:(hl + 1) * 64, hp,
                                  g * 512 + js[d] * 128:(g + 1) * 512],
                        start=True, stop=True,
                    )
            for hl in range(2):
                et = epool.tile([128, 1024], bf16, tag="E")
                nc.scalar.activation(
                    et[:, 0:cb[1] + ws[1]], pss[hl][:, 0:cb[1] + ws[1]],
                    AF.Exp, scale=scale,
                )
                # mask the diagonal 128-block of each band chunk
                for d in range(2):
                    if kcs[d] >= c.QT * g:
                        nc.vector.tensor_tensor(
                            out=et[:, cb[d]:cb[d] + 128],
                            in0=et[:, cb[d]:cb[d] + 128],
                            in1=tri01[:], op=ALU.mult,
                        )
                etiles[(hl, kp)] = (et, cb)

    def pv_group(hp, g, etiles):
        """pv + normalization for (hp, g); etiles from attn_group(hp, g)."""
        kmax = (g + 1) * c.QT
        for hl in range(2):
            h = 2 * hp + hl
            psv = ps_pv.tile([128, 512], f32, tag="pv")
            for kc in range(kmax):
                kp, d = divmod(kc, 2)
                et, cb = etiles[(hl, kp)]
                j = jstart(kc, g)
                nc.tensor.matmul(
                    psv[:, j * 128:512],
                    lhsT=v_sb[:, kc, h, :],
                    rhs=et[:, cb[d]:cb[d] + 512 - 128 * j],
                    start=(kc == 0), stop=(kc == kmax - 1),
                )
            # softmax denominators sit in psum row 64; reciprocal there,
            # broadcast down 64 partitions via a stride-0 DMA, multiply.
            if dbg and hp == 0 and g == 0 and hl == 0:
                dpsv = stg.tile([128, 512], f32, tag="dpsv")
                nc.vector.tensor_copy(dpsv[:], psv[:])
                nc.sync.dma_start(out=dbg["psv"], in_=dpsv[:])
                nc.sync.dma_start(out=dbg["E00"][:, 0:896],
                                  in_=etiles[(0, 0)][0][:, 0:896])
            rf = rfpool.tile([1, 512], f32, tag="rf")
            nc.vector.reciprocal_approx_fast(out=rf[:], in_=psv[0:1, :])
            rb = rbpool.tile([128, 512], f32, tag="rb")
            s0 = rf[0:1, :]
            src = bass.AP(s0.tensor, s0.offset,
                          [list(s0.ap[0]), [0, 64], list(s0.ap[1])])
            nc.sync.dma_start(out=rb[64:128, :], in_=src)
            if dbg and hp == 0 and g == 0 and hl == 0:
                nc.sync.dma_start(out=dbg["rf"], in_=rf[:])
                nc.sync.dma_start(out=dbg["rb"], in_=rb[64:128, :])
            if hl == 1:
                nc.vector.tensor_tensor(
                    out=aT_sb[64:128, hp, g * 512:(g + 1) * 512],
                    in0=psv[64:128, :], in1=rb[64:128, :], op=ALU.mult,
                )
            else:
                st = stg.tile([128, 512], bf16, tag="stg")
                nc.vector.tensor_tensor(
                    out=st[64:128, :], in0=psv[64:128, :], in1=rb[64:128, :],
                    op=ALU.mult,
                )
                nc.gpsimd.dma_start(
                    out=aT_sb[0:64, hp, g * 512:(g + 1) * 512],
                    in_=st[64:128, :],
                )

    ostage = ctx.enter_context(tc.tile_pool(name="ostage", bufs=4))

    def emit_outproj(g):
        """Out-projection rows for q-group g (t-chunks 4g..4g+3)."""
        for t in range(g * c.QT, (g + 1) * c.QT):
            ps = ps_mm.tile([128, 1024], f32, tag="mm")
            for d in range(2):
                for ci in range(c.MC):
                    nc.tensor.matmul(
                        ps[:, d * 512:(d + 1) * 512],
                        lhsT=aT_sb[:, ci, t * 128:(t + 1) * 128],
                        rhs=wo_sb[:, ci, d * 512:(d + 1) * 512],
                        start=(ci == 0), stop=(ci == c.MC - 1),
                    )
            ot = ostage.tile([128, 1024], bf16, tag="o")
            if t % 2 == 0:
                nc.vector.tensor_copy(ot[:], ps[:])
            else:
                nc.scalar.copy(ot[:], ps[:])
            nc.sync.dma_start(out=out[t * 128:(t + 1) * 128, :], in_=ot[:])

    # ---- emission schedule ----
    emit_qk_tile(0, "q", 0)
    emit_qk_tile(0, "k", 0)
    emit_qk_tile(0, "q", 1)
    emit_qk_tile(0, "k", 1)
    emit_v_tile(0)

    fillers = [lambda tg=tg: emit_v_tile(tg) for tg in range(1, c.TC // VG)]
    for n in range(c.T // W2):
        fillers.append(lambda n=n: emit_qk_tile(1, "q", n))
        fillers.append(lambda n=n: emit_qk_tile(1, "k", n))

    # head-pair 0: scores(g) then pv(g-1); fillers ride between groups
    et_prev, et_cur = None, {}
    for g in range(c.QG):
        attn_group(0, g, et_cur)
        if fillers:
            fillers.pop(0)()
        if fillers:
            fillers.pop(0)()
        if g > 0:
            pv_group(0, g - 1, et_prev)
        et_prev, et_cur = et_cur, {}
    while fillers:
        fillers.pop(0)()
    pv_group(0, c.QG - 1, et_prev)

    # head-pair 1: scores(g+1), pv(g), outproj(g)
    et_prev, et_cur = None, {}
    for g in range(c.QG):
        attn_group(1, g, et_cur)
        if g > 0:
            pv_group(1, g - 1, et_prev)
            emit_outproj(g - 1)
        et_prev, et_cur = et_cur, {}
    pv_group(1, c.QG - 1, et_prev)
    emit_outproj(c.QG - 1)
    if dbg:
        nc.sync.dma_start(out=dbg["qT"], in_=qT_sb[:])
        nc.sync.dma_start(out=dbg["kT"], in_=kT_sb[:])
        nc.sync.dma_start(out=dbg["v"], in_=v_sb[:])
        nc.sync.dma_start(out=dbg["aT"], in_=aT_sb[:])


# ---------------------------------------------------------------------------
# host side
# ---------------------------------------------------------------------------

_CACHE: dict = {}


def _get_program(cfg: Cfg):
    if cfg not in _CACHE:
        _CACHE[cfg] = build_program(cfg)
    return _CACHE[cfg]


def _mask_mode(mask: np.ndarray, T: int) -> str:
    m = (np.asarray(mask).reshape(T, T) != 0)
    if m.all():
        return "full"
    if np.array_equal(m, np.tril(np.ones((T, T), dtype=bool))):
        return "causal"
    return "bias"


def make_in_maps(cfg: Cfg, x, W_qkv, b_qkv, W_out, mask=None):
    c = cfg
    npmm = c.npmm
    B = x.shape[0]
    n_hg = N_CORES // B
    xTs = []
    for b in range(B):
        xa = np.zeros((c.DM + 128, c.T), dtype=npmm)
        xa[:c.DM] = np.ascontiguousarray(x[b].T).astype(npmm)
        xa[c.DM] = npmm(1.0)
        xTs.append(xa)
    in_maps = []
    for core in range(N_CORES):
        b, hg = divmod(core, n_hg)
        col0 = hg * c.NHD
        wq_ = np.ascontiguousarray(
            W_qkv[:, col0:col0 + c.NHD]).astype(npmm)
        wk_ = np.ascontiguousarray(
            W_qkv[:, c.DM + col0:c.DM + col0 + c.NHD]).astype(npmm)
        wv_ = np.zeros((c.DM + 128, c.NHD), dtype=npmm)
        wv_[:c.DM] = W_qkv[:, 2 * c.DM + col0:2 * c.DM + col0 + c.NHD].astype(npmm)
        wv_[c.DM] = b_qkv[2 * c.DM + col0:2 * c.DM + col0 + c.NHD].astype(npmm)
        bq_ = np.ascontiguousarray(
            b_qkv[col0:col0 + c.NHD].reshape(c.MC, 128).T).astype(np.float32)
        bk_ = np.ascontiguousarray(
            b_qkv[c.DM + col0:c.DM + col0 + c.NHD].reshape(c.MC, 128).T
        ).astype(np.float32)
        wo_ = np.ascontiguousarray(W_out[col0:col0 + c.NHD, :]).astype(npmm)
        in_maps.append(dict(xT=xTs[b], wq=wq_, wk=wk_, wv=wv_, bq=bq_,
                            bk=bk_, wo=wo_))
    return in_maps


def run_sharded(cfg: Cfg, x, W_qkv, b_qkv, W_out, b_out, mask=None, **kw):
    nc, _names = _get_program(cfg)
    in_maps = make_in_maps(cfg, x, W_qkv, b_qkv, W_out, mask)
    res = bass_utils.run_bass_kernel_spmd(
        nc, in_maps, core_ids=list(range(N_CORES)), **kw,
    )
    outs = [np.asarray(r["out"], dtype=np.float32) for r in res.results]
    B = x.shape[0]
    n_hg = N_CORES // B
    y = np.stack([
        np.sum(outs[b * n_hg:(b + 1) * n_hg], axis=0) for b in range(B)
    ]) + b_out.astype(np.float32)
    return y.astype(np.float32), res


def kernel(x, W_qkv, b_qkv, W_out, b_out, mask):
    x = np.asarray(x, dtype=np.float32)
    W_qkv = np.asarray(W_qkv, dtype=np.float32)
    b_qkv = np.asarray(b_qkv, dtype=np.float32)
    W_out = np.asarray(W_out, dtype=np.float32)
    b_out = np.asarray(b_out, dtype=np.float32)
    B, T, DM = x.shape
    mode = _mask_mode(mask, T)
    cfg = Cfg(T=T, DM=DM, mode=mode)
    y, _ = run_sharded(cfg, x, W_qkv, b_qkv, W_out, b_out, mask)
    return y


# revision 16
# speedup vs baseline: 1.0742x; 1.0742x over previous
"""Multi-head attention (B=2, T=2048, d_model=1024, H=16, hd=64) on 8 Trainium2
NeuronCores.

Sharding: 4 consecutive heads of one batch per core (core c -> batch c//4,
heads 4*(c%4)..+3). Each core: QKV projection slice, causal attention, partial
out-projection (its 256 rows of W_out); host sums 4 partials/batch + b_out.

v2 layout (everything stays transposed end-to-end; no on-chip transposes):
  qT/kT [hd, T]   <- lhsT=W chunk, rhs=xT (2 heads per 128 partitions)
  sT    [k, q]    <- two K=64 matmuls, one per head, row-paired on the PE
                     (tile_position rows 0/64) -> concurrent
  E     [k, q]    <- exp(sT/8) on ScalarE (bf16); diagonal blocks masked
                     post-exp with a 0/1 triangle; above-diagonal blocks never
                     computed (scores/exp/pv all narrowed to the causal band)
  pvT   [65, q]   <- lhsT=[v|1] chunk, rhs=E chunk: rows 0-63 unnormalized
                     aT, row 64 = softmax denominators (free)
  aT    [hd, q]   = pvT[0:64] * bcast(1/pvT[64]) (recip_approx_fast on the
                     psum row + stride-0 broadcast DMA + one DVE multiply)
  out  += aT.T @ W_out chunk  (psum -> DRAM DMA directly, fp32)

Emission is software-pipelined: head-pair 0's attention starts right after
its projections (pair-1 projections + v tiles ride along as fillers); pv lags
scores by one q-group so the PE never stalls on exp; out-projection for
q-group g is emitted inside head-pair 1's loop right after group g finishes.
"""

import math
import os
from contextlib import ExitStack
from dataclasses import dataclass

import numpy as np
import ml_dtypes

import concourse.bass as bass
import concourse.tile as tile
from concourse import bacc, mybir
from concourse import bass_utils

AF = mybir.ActivationFunctionType
ALU = mybir.AluOpType
DT = mybir.dt

N_CORES = 8


@dataclass(frozen=True)
class Cfg:
    T: int = 2048        # sequence length
    DM: int = 1024       # d_model
    HD: int = 64         # head dim
    NH: int = 4          # heads per core
    mode: str = "causal"
    mm: str = "bf16"
    dbg: bool = False

    @property
    def NHD(self):
        return self.NH * self.HD          # qkv slice width per core

    @property
    def KC(self):
        return self.DM // 128             # contraction chunks for projections

    @property
    def MC(self):
        return self.NHD // 128            # head-pair chunks

    @property
    def TC(self):
        return self.T // 128              # t chunks

    @property
    def QW(self):
        return 512                        # q group width

    @property
    def QG(self):
        return self.T // self.QW

    @property
    def QT(self):
        return self.QW // 128             # q tiles (128) per group

    @property
    def npmm(self):
        return ml_dtypes.bfloat16


def build_program(cfg: Cfg):
    c = cfg
    assert c.mode == "causal"
    assert c.DM == 1024 and c.T == 2048 and c.HD == 64 and c.NH == 4
    nc = bacc.Bacc("TRN2", target_bir_lowering=False, debug=False,
                   num_devices=N_CORES)
    f32 = DT.float32
    bf16 = DT.bfloat16

    # xT has an extra 128-row chunk: row 1024 = ones (for the v bias matmul)
    xT = nc.dram_tensor("xT", [c.DM + 128, c.T], bf16, kind="ExternalInput").ap()
    wq = nc.dram_tensor("wq", [c.DM, c.NHD], bf16, kind="ExternalInput").ap()
    wk = nc.dram_tensor("wk", [c.DM, c.NHD], bf16, kind="ExternalInput").ap()
    # wv has the bias as row 1024 (contracts against the ones row of xT)
    wv = nc.dram_tensor("wv", [c.DM + 128, c.NHD], bf16, kind="ExternalInput").ap()
    bq = nc.dram_tensor("bq", [128, c.MC], f32, kind="ExternalInput").ap()
    bk = nc.dram_tensor("bk", [128, c.MC], f32, kind="ExternalInput").ap()
    wo = nc.dram_tensor("wo", [c.NHD, c.DM], bf16, kind="ExternalInput").ap()
    out = nc.dram_tensor("out", [c.T, c.DM], bf16, kind="ExternalOutput").ap()
    dbg = {}
    if c.dbg:
        dbg["qT"] = nc.dram_tensor("dbg_qT", [128, c.MC, c.T], bf16, kind="ExternalOutput").ap()
        dbg["kT"] = nc.dram_tensor("dbg_kT", [128, c.MC, c.T], bf16, kind="ExternalOutput").ap()
        dbg["v"] = nc.dram_tensor("dbg_v", [128, c.TC, c.NH, 128], bf16, kind="ExternalOutput").ap()
        dbg["aT"] = nc.dram_tensor("dbg_aT", [128, c.MC, c.T], bf16, kind="ExternalOutput").ap()

    with tile.TileContext(nc) as tc, ExitStack() as ctx:
        _body(ctx, tc, c, xT, wq, wk, wv, bq, bk, wo, out, dbg)
    nc.compile()
    return nc, ["xT", "wq", "wk", "wv", "bq", "bk", "wo"]


def _body(ctx, tc, c: Cfg, xT, wq, wk, wv, bq, bk, wo, out, dbg=None):
    nc = tc.nc
    f32 = DT.float32
    bf16 = DT.bfloat16
    scale = 1.0 / math.sqrt(c.HD)
    HD1 = c.HD + 1

    const = ctx.enter_context(tc.tile_pool(name="const", bufs=1))
    big = ctx.enter_context(tc.tile_pool(name="big", bufs=1))
    epool = ctx.enter_context(tc.tile_pool(name="E", bufs=30))
    rfpool = ctx.enter_context(tc.tile_pool(name="rf", bufs=2))
    rbpool = ctx.enter_context(tc.tile_pool(name="rb", bufs=2))
    stg = ctx.enter_context(tc.tile_pool(name="stg", bufs=2))
    # PSUM: 3 x [128,1024] (6 banks) + 2 x [65,512] (2 banks)
    ps_mm = ctx.enter_context(tc.tile_pool(name="psmm", bufs=3, space="PSUM"))
    ps_pv = ctx.enter_context(tc.tile_pool(name="pspv", bufs=2, space="PSUM"))

    # ---- input DMAs (ordered for earliest compute start) ----
    bq_sb = const.tile([128, c.MC], f32, tag="bq")
    nc.sync.dma_start(out=bq_sb[:], in_=bq)
    bk_sb = const.tile([128, c.MC], f32, tag="bk")
    nc.sync.dma_start(out=bk_sb[:], in_=bk)

    wq_sb = big.tile([128, c.KC, c.NHD], bf16, tag="wq")
    nc.sync.dma_start(out=wq_sb[:], in_=wq.rearrange("(c p) n -> p c n", p=128))
    wk_sb = big.tile([128, c.KC, c.NHD], bf16, tag="wk")
    nc.sync.dma_start(out=wk_sb[:], in_=wk.rearrange("(c p) n -> p c n", p=128))

    XC = c.KC + 1
    xT_sb = big.tile([128, XC, c.T], bf16, tag="xT")
    xTd = xT.rearrange("(c p) t -> p c t", p=128)
    TH = c.T // 2
    nc.sync.dma_start(out=xT_sb[:, :, 0:TH], in_=xTd[:, :, 0:TH])

    wv_sb = big.tile([128, XC, c.NHD], bf16, tag="wv")
    nc.sync.dma_start(out=wv_sb[:], in_=wv.rearrange("(c p) n -> p c n", p=128))

    nc.sync.dma_start(out=xT_sb[:, :, TH:c.T], in_=xTd[:, :, TH:c.T])

    wo_sb = big.tile([128, c.MC, c.DM], bf16, tag="wo")
    nc.sync.dma_start(out=wo_sb[:], in_=wo.rearrange("(c p) n -> p c n", p=128))

    # 0/1 lower-causal triangle: tri01[k, q] = 1 if q >= k else 0
    tri01 = const.tile([128, 128], bf16, tag="tri")
    nc.gpsimd.memset(tri01[:], 1.0)
    nc.gpsimd.affine_select(
        out=tri01[:], in_=tri01[:],
        compare_op=ALU.is_ge, fill=0.0,
        base=0, channel_multiplier=-1, pattern=[[1, 128]],
    )

    # ---- projection targets ----
    qT_sb = big.tile([128, c.MC, c.T], bf16, tag="qT")
    kT_sb = big.tile([128, c.MC, c.T], bf16, tag="kT")
    v_sb = big.tile([128, c.TC, c.NH, 128], bf16, tag="v")
    nc.vector.memset(v_sb[:, :, :, 0:c.HD], 0.0)
    nc.vector.memset(v_sb[:, :, :, 0:1], 1.0)
    aT_sb = big.tile([128, c.MC, c.T], bf16, tag="aT")

    W2 = 1024                              # q/k psum tile width (t cols)
    VG = 4                                 # t-chunks per v psum tile

    def emit_qk_tile(m, which, n):
        """One [128, 1024] psum tile of the q or k projection."""
        w_sb, b_sb, dst = ((wq_sb, bq_sb, qT_sb) if which == "q"
                           else (wk_sb, bk_sb, kT_sb))
        ps = ps_mm.tile([128, 1024], f32, tag="mm")
        for d in range(W2 // 512):
            for kc in range(c.KC):
                nc.tensor.matmul(
                    ps[:, d * 512:(d + 1) * 512],
                    lhsT=w_sb[:, kc, m * 128:(m + 1) * 128],
                    rhs=xT_sb[:, kc, n * W2 + d * 512:n * W2 + (d + 1) * 512],
                    start=(kc == 0), stop=(kc == c.KC - 1),
                )
        nc.vector.tensor_scalar_add(
            dst[:, m, n * W2:(n + 1) * W2], ps[:, 0:W2], b_sb[:, m:m + 1],
        )

    def emit_v_tile(tg):
        """VG t-chunks of v for all heads (+ bias via the ones row chunk)."""
        ps = ps_mm.tile([128, 1024], f32, tag="mm")
        for d in range(VG):
            t = tg * VG + d
            for kc in range(XC):
                nc.tensor.matmul(
                    ps[:, d * c.NHD:(d + 1) * c.NHD],
                    lhsT=xT_sb[:, kc, t * 128:(t + 1) * 128],
                    rhs=wv_sb[:, kc, :],
                    start=(kc == 0), stop=(kc == XC - 1),
                )
        for d in range(VG):
            t = tg * VG + d
            nc.vector.tensor_copy(
                v_sb[:, t, :, c.HD:128],
                ps[:, d * c.NHD:(d + 1) * c.NHD].rearrange(
                    "p (h d) -> p h d", d=c.HD),
            )

    # ---- attention ----
    # Per (hp, g): kc ranges over the causal band 0..4g+3. For band chunks
    # (kc >= 4g) only columns q >= kc*128 exist anywhere: scores, exp and pv
    # are all narrowed. One [128,1024] psum tile per kc packs both heads of
    # the pair side by side [h0 w | h1 w] -> one ACT covers both.
    def jstart(kc, g):
        return max(0, kc - c.QT * g)

    # Rate-matched feeder: between score rounds the emitter interleaves
    # small work items (pv matmuls, normalization, out-projection tiles,
    # projection fillers) so the PE stays busy while ScalarE drains the exp
    # backlog, without running further ahead than the psum FIFO allows.
    # "urgent" items (pv/norm/outproj) free psum/E-pool slots and pop before
    # "bulk" fillers. Costs are PE-time estimates in ns.
    feed_urgent = []
    feed_state = {"credit": 0.0}

    def feed(budget_ns):
        feed_state["credit"] += budget_ns
        while feed_state["credit"] > 0 and feed_urgent:
            cost, fn = feed_urgent.pop(0)
            fn()
            feed_state["credit"] -= cost

    def force(cost_ns, fn):
        """Emit a prerequisite blob now; charge its PE cost to the credit."""
        fn()
        feed_state["credit"] -= cost_ns

    def flush_feed():
        while feed_urgent:
            _, fn = feed_urgent.pop(0)
            fn()
        feed_state["credit"] = 0.0

    def attn_group(hp, g, etiles):
        for kc in range((g + 1) * c.QT):
            j = jstart(kc, g)
            w = 512 - 128 * j
            cb = 512 - w          # anchor the pair at the psum bank seam
            ps = ps_mm.tile([128, 1024], f32, tag="mm")
            for hl in range(2):
                nc.tensor.matmul(
                    ps[:, cb + hl * w:cb + (hl + 1) * w],
                    lhsT=kT_sb[hl * 64:(hl + 1) * 64, hp,
                               kc * 128:(kc + 1) * 128],
                    rhs=qT_sb[hl * 64:(hl + 1) * 64, hp,
                              g * 512 + j * 128:(g + 1) * 512],
                    start=True, stop=True,
                )
            et = epool.tile([128, 1024], bf16, tag="E")
            nc.scalar.activation(et[:, cb:cb + 2 * w], ps[:, cb:cb + 2 * w],
                                 AF.Exp, scale=scale)
            if kc >= c.QT * g:
                # mask the diagonal 128-block of each head's range
                for hl in range(2):
                    nc.vector.tensor_tensor(
                        out=et[:, cb + hl * w:cb + hl * w + 128],
                        in0=et[:, cb + hl * w:cb + hl * w + 128],
                        in1=tri01[:], op=ALU.mult,
                    )
            etiles[kc] = (et, w)
            feed((2 * w + 352) / 1.2 - (w / 2.4 + 19))

    def queue_pv(hp, g, etiles):
        """Queue pv + normalization work items for (hp, g)."""
        kmax = (g + 1) * c.QT
        for hl in range(2):
            h = 2 * hp + hl
            box = {}

            def pv_mm(kc, hl=hl, h=h, box=box, etiles=etiles, g=g,
                      kmax=kmax):
                if kc == 0:
                    psv_t = ps_pv.tile([128, 512], f32, tag="pv")
                    box["psv"] = psv_t
                psv = box["psv"]
                et, w = etiles[kc]
                j = jstart(kc, g)
                cb = 512 - w
                nc.tensor.matmul(
                    psv[:, j * 128:512],
                    lhsT=v_sb[:, kc, h, :],
                    rhs=et[:, cb + hl * w:cb + (hl + 1) * w],
                    start=(kc == 0), stop=(kc == kmax - 1),
                )

            for kc in range(kmax):
                w = 512 - 128 * jstart(kc, g)
                feed_urgent.append((w / 2.4 + 10,
                                    lambda kc=kc, f=pv_mm: f(kc)))

            def norm(hl=hl, hp=hp, g=g, box=box):
                # softmax denominators sit in psum row 0 (ones column of
                # the [1|0*63|v] lhsT); aT in rows 64-127. Reciprocal on
                # row 0, broadcast down via a stride-0 DMA, one multiply.
                psv = box["psv"]
                rf = rfpool.tile([1, 512], f32, tag="rf")
                nc.vector.reciprocal_approx_fast(out=rf[:], in_=psv[0:1, :])
                rb = rbpool.tile([128, 512], f32, tag="rb")
                s0 = rf[0:1, :]
                src = bass.AP(s0.tensor, s0.offset,
                              [list(s0.ap[0]), [0, 64], list(s0.ap[1])])
                nc.sync.dma_start(out=rb[64:128, :], in_=src)
                if hl == 1:
                    nc.vector.tensor_tensor(
                        out=aT_sb[64:128, hp, g * 512:(g + 1) * 512],
                        in0=psv[64:128, :], in1=rb[64:128, :], op=ALU.mult,
                    )
                else:
                    st = stg.tile([128, 512], bf16, tag="stg")
                    nc.vector.tensor_tensor(
                        out=st[64:128, :], in0=psv[64:128, :],
                        in1=rb[64:128, :], op=ALU.mult,
                    )
                    nc.gpsimd.dma_start(
                        out=aT_sb[0:64, hp, g * 512:(g + 1) * 512],
                        in_=st[64:128, :],
                    )

            feed_urgent.append((0, norm))

    ostage = ctx.enter_context(tc.tile_pool(name="ostage", bufs=4))

    def queue_outproj(g):
        for t in range(g * c.QT, (g + 1) * c.QT):
            def op_t(t=t):
                ps = ps_mm.tile([128, 1024], f32, tag="mm")
                for d in range(2):
                    for ci in range(c.MC):
                        nc.tensor.matmul(
                            ps[:, d * 512:(d + 1) * 512],
                            lhsT=aT_sb[:, ci, t * 128:(t + 1) * 128],
                            rhs=wo_sb[:, ci, d * 512:(d + 1) * 512],
                            start=(ci == 0), stop=(ci == c.MC - 1),
                        )
                ot = ostage.tile([128, 1024], bf16, tag="o")
                if t % 2 == 0:
                    nc.vector.tensor_copy(ot[:], ps[:])
                else:
                    nc.scalar.copy(ot[:], ps[:])
                nc.sync.dma_start(out=out[t * 128:(t + 1) * 128, :],
                                  in_=ot[:])
            feed_urgent.append((900, op_t))

    # ---- emission schedule ----
    # Prerequisite projection tiles are force-emitted just before the first
    # group whose scores/pv read them (emission order is program order for
    # the dependency tracker); pv/norm/outproj items are credit-fed between
    # score rounds so the PE never idles behind the exp backlog.
    emit_qk_tile(0, "q", 0)
    emit_qk_tile(0, "k", 0)
    emit_v_tile(0)

    prereqs = {
        (0, 2): [(3400, lambda: emit_qk_tile(0, "q", 1)),
                 (3400, lambda: emit_qk_tile(0, "k", 1)),
                 (3900, lambda: emit_v_tile(1))],
        (0, 3): [(3900, lambda: emit_v_tile(2))],
        (1, 0): [(3900, lambda: emit_v_tile(3)),
                 (3400, lambda: emit_qk_tile(1, "q", 0)),
                 (3400, lambda: emit_qk_tile(1, "k", 0))],
        (1, 2): [(3400, lambda: emit_qk_tile(1, "q", 1)),
                 (3400, lambda: emit_qk_tile(1, "k", 1))],
    }

    ets = {}
    for hp in range(2):
        for g in range(c.QG):
            for cost, fn in prereqs.get((hp, g), ()):
                force(cost, fn)
            ets[(hp, g)] = {}
            attn_group(hp, g, ets[(hp, g)])
            queue_pv(hp, g, ets[(hp, g)])
            if hp == 1:
                queue_outproj(g)
    flush_feed()

    if dbg:
        nc.sync.dma_start(out=dbg["qT"], in_=qT_sb[:])
        nc.sync.dma_start(out=dbg["kT"], in_=kT_sb[:])
        nc.sync.dma_start(out=dbg["v"], in_=v_sb[:])
        nc.sync.dma_start(out=dbg["aT"], in_=aT_sb[:])


# ---------------------------------------------------------------------------
# host side
# ---------------------------------------------------------------------------

_CACHE: dict = {}


def _get_program(cfg: Cfg):
    if cfg not in _CACHE:
        _CACHE[cfg] = build_program(cfg)
    return _CACHE[cfg]


def _mask_mode(mask: np.ndarray, T: int) -> str:
    m = (np.asarray(mask).reshape(T, T) != 0)
    if m.all():
        return "full"
    if np.array_equal(m, np.tril(np.ones((T, T), dtype=bool))):
        return "causal"
    return "bias"


def make_in_maps(cfg: Cfg, x, W_qkv, b_qkv, W_out, mask=None):
    c = cfg
    npmm = c.npmm
    B = x.shape[0]
    n_hg = N_CORES // B
    xTs = []
    for b in range(B):
        xa = np.zeros((c.DM + 128, c.T), dtype=npmm)
        xa[:c.DM] = np.ascontiguousarray(x[b].T).astype(npmm)
        xa[c.DM] = npmm(1.0)
        xTs.append(xa)
    in_maps = []
    for core in range(N_CORES):
        b, hg = divmod(core, n_hg)
        col0 = hg * c.NHD
        wq_ = np.ascontiguousarray(
            W_qkv[:, col0:col0 + c.NHD]).astype(npmm)
        wk_ = np.ascontiguousarray(
            W_qkv[:, c.DM + col0:c.DM + col0 + c.NHD]).astype(npmm)
        wv_ = np.zeros((c.DM + 128, c.NHD), dtype=npmm)
        wv_[:c.DM] = W_qkv[:, 2 * c.DM + col0:2 * c.DM + col0 + c.NHD].astype(npmm)
        wv_[c.DM] = b_qkv[2 * c.DM + col0:2 * c.DM + col0 + c.NHD].astype(npmm)
        bq_ = np.ascontiguousarray(
            b_qkv[col0:col0 + c.NHD].reshape(c.MC, 128).T).astype(np.float32)
        bk_ = np.ascontiguousarray(
            b_qkv[c.DM + col0:c.DM + col0 + c.NHD].reshape(c.MC, 128).T
        ).astype(np.float32)
        wo_ = np.ascontiguousarray(W_out[col0:col0 + c.NHD, :]).astype(npmm)
        in_maps.append(dict(xT=xTs[b], wq=wq_, wk=wk_, wv=wv_, bq=bq_,
                            bk=bk_, wo=wo_))
    return in_maps


def run_sharded(cfg: Cfg, x, W_qkv, b_qkv, W_out, b_out, mask=None, **kw):
    nc, _names = _get_program(cfg)
    in_maps = make_in_maps(cfg, x, W_qkv, b_qkv, W_out, mask)
    res = bass_utils.run_bass_kernel_spmd(
        nc, in_maps, core_ids=list(range(N_CORES)), **kw,
    )
    outs = [np.asarray(r["out"], dtype=np.float32) for r in res.results]
    B = x.shape[0]
    n_hg = N_CORES // B
    y = np.stack([
        np.sum(outs[b * n_hg:(b + 1) * n_hg], axis=0) for b in range(B)
    ]) + b_out.astype(np.float32)
    return y.astype(np.float32), res


def kernel(x, W_qkv, b_qkv, W_out, b_out, mask):
    x = np.asarray(x, dtype=np.float32)
    W_qkv = np.asarray(W_qkv, dtype=np.float32)
    b_qkv = np.asarray(b_qkv, dtype=np.float32)
    W_out = np.asarray(W_out, dtype=np.float32)
    b_out = np.asarray(b_out, dtype=np.float32)
    B, T, DM = x.shape
    mode = _mask_mode(mask, T)
    cfg = Cfg(T=T, DM=DM, mode=mode)
    y, _ = run_sharded(cfg, x, W_qkv, b_qkv, W_out, b_out, mask)
    return y
